# revision 14
# baseline (speedup 1.0000x reference)
"""Trainium2 Bass kernel for nn_CrossAttention_47502338294587.

Math: the reference cross-attention has a single KV position broadcast over
all T query positions.  Softmax over a row of identical logits is uniform,
so attention output == v for every query, and the whole module collapses to

    out[b, t, :] = (visual_features[b] @ Wv + bv) @ Wp + bp      (for all t)

independent of x / Wq / Wk.  The device computes the two projections and
broadcasts the per-batch row over the T axis; the host only does input
layout prep (incl. bf16 weight packing) and shard re-assembly.

Sharding: tensor-parallel over the output channel dim C - core i computes
and writes out[:, :, i*128:(i+1)*128] (full Wv, column shard of Wp / bp).

Pipeline (per core):
  Wv streams as column blocks wv_j = Wv[:, j*128:(j+1)*128] packed
  [p, k, c] = Wv[k*128+p, j*128+c].  Only 10 input DMAs total (tiny
  consts first, then {0,2} and {1,3} paired 512KB transfers, then 4
  single blocks) so DMA-completion semaphores (8 lanes) never stall an
  issue against an in-flight transfer.  As each block lands the PE
  computes the transposed projection directly:

    psum_vvT_j[c', b] = bv_j (K=1 opener) + sum_k wv_j[:,k,:]^T @ vfT_k

  (no 4-partition PSUM casts, no PE transposes).  A [128,4] DVE copy
  drops vvT_j to SBUF and one accumulating matmul folds it into the
  output row while later blocks still stream:

    psum_row[b, ci] = bp (K=1 opener) + sum_j vvT_j^T @ Wp_j

  The broadcast uses a constant one-hot mask (no DVE multiply):

    pbcT[ci, (tc, b')] = sum_b prow[b, ci] * mask[b, (tc, b')],
    mask[b, (tc,b')] = (b == b')

  giving the [128, 512] bf16 tile replicated by the out DMA over the 8
  T-chunks, split across both HWDGE queues.  Out layout is
  out[p=ci_local, q, tc, b] (1KB contiguous runs); the host transposes
  and upcasts to f32 during re-assembly.
"""

import os
import sys

import numpy as np

for _p in ("/opt/trn_rl_repo",):
    if _p not in sys.path and os.path.isdir(_p):
        sys.path.insert(0, _p)

B, T, C = 4, 1024, 1024
N_CORES = 8
CSH = C // N_CORES  # 128, C-shard per core
KC = C // 128  # 8 contraction chunks

_BUILT = None


def build_nc():
    """Build + compile the Bass program (one NeuronCore's SPMD body)."""
    import concourse.bass as bass
    import concourse.mybir as mybir
    import concourse.tile as tile
    from concourse import bacc

    f32 = mybir.dt.float32
    bf16 = mybir.dt.bfloat16
    nc = bacc.Bacc("TRN2", target_bir_lowering=False, debug=False)

    # ---- DRAM inputs (host pre-packed layouts) --------------------------
    # paired blocks {0,2} and {1,3}: [p, pair, k, c]; singles 4..7: [p, k, c]
    wv02_d = nc.dram_tensor("wv02", [128, 2, KC, 128], bf16, kind="ExternalInput")
    wv13_d = nc.dram_tensor("wv13", [128, 2, KC, 128], bf16, kind="ExternalInput")
    wvs_d = {
        j: nc.dram_tensor(f"wv{j}", [128, KC, 128], bf16, kind="ExternalInput")
        for j in (4, 5, 6, 7)
    }
    # vft[p, k*4 + b] = vf[b, k*128 + p]
    vft_d = nc.dram_tensor("vft", [128, 32], bf16, kind="ExternalInput")
    # wp halves: wpa[p, j, c] = Wp[j*128 + p, ci_c] for j in 0..3; wpb j 4..7
    wpa_d = nc.dram_tensor("wpa", [128, 4, CSH], bf16, kind="ExternalInput")
    wpb_d = nc.dram_tensor("wpb", [128, 4, CSH], bf16, kind="ExternalInput")
    # hdr2 rows 0-4:
    #   [0, 0:1024]      bv
    #   [0:4, 1024:1536] mask[b, tc*4 + b'] = (b == b')
    #   [0, 1536:1664]   bp[ci]
    #   [0, 1664:1668]   ones4
    hdr2_d = nc.dram_tensor("hdr2", [5, 1668], bf16, kind="ExternalInput")
    # out[p, q, tc, b] = out_full[b, q*128 + tc, ci_p]  (1KB runs per q)
    out = nc.dram_tensor("out", [128, KC, 128, B], bf16, kind="ExternalOutput")

    with tile.TileContext(nc) as tc:
        with (
            tc.tile_pool(name="sb", bufs=1) as sb,
            tc.tile_pool(name="pv", bufs=1, space="PSUM") as pv,
            tc.tile_pool(name="pr", bufs=1, space="PSUM") as pr,
            tc.tile_pool(name="pb", bufs=1, space="PSUM") as pb,
        ):
            # ---- SBUF tiles -------------------------------------------------
            wv02_t = sb.tile([128, 2, KC, 128], bf16, tag="wv02")
            wv13_t = sb.tile([128, 2, KC, 128], bf16, tag="wv13")
            wvs_t = {
                j: sb.tile([128, KC, 128], bf16, name=f"wv{j}", tag=f"wv{j}")
                for j in (4, 5, 6, 7)
            }
            vft_t = sb.tile([128, 32], bf16, tag="vft")
            wpa_t = sb.tile([128, 4, CSH], bf16, tag="wpa")
            wpb_t = sb.tile([128, 4, CSH], bf16, tag="wpb")
            hdr2_t = sb.tile([5, 1668], bf16, tag="hdr2")
            vvt_sb = [
                sb.tile([128, B], bf16, name=f"vvt{j}", tag=f"vvt{j}")
                for j in range(KC)
            ]
            bc_t = sb.tile([128, 128 * B], bf16, tag="bc")

            vft = vft_t[:].rearrange("p (k b) -> p k b", b=B)
            bv_row = hdr2_t[0:1, 0:1024]
            mask4 = hdr2_t[0:4, 1024:1536]
            bp_row = hdr2_t[0:1, 1536:1664]
            ones4 = hdr2_t[0:1, 1664:1668]

            def wv_block(j):
                if j == 0:
                    return wv02_t[:, 0]
                if j == 2:
                    return wv02_t[:, 1]
                if j == 1:
                    return wv13_t[:, 0]
                if j == 3:
                    return wv13_t[:, 1]
                return wvs_t[j][:]

            # ---- PSUM tiles -------------------------------------------------
            psum_vvt = [
                pv.tile([128, B], f32, name=f"pvt{h}", tag=f"pvt{h}")
                for h in range(3)
            ]
            psum_row = pr.tile([B, CSH], f32, tag="pr")
            psum_bc = pb.tile([128, 128 * B], f32, tag="pb")

            # ---- DMA in (balanced dual HWDGE queues, smalls first) ----------
            # sync:   hdr2, wpa, wv{0,2}, wv4, wv6      (1.17 MB, 5 DMAs)
            # scalar: vft, wpb, wv{1,3}, wv5, wv7       (1.16 MB, 5 DMAs)
            nc.sync.dma_start(hdr2_t[:], hdr2_d[:, :])
            nc.scalar.dma_start(vft_t[:], vft_d[:, :])
            nc.sync.dma_start(wpa_t[:], wpa_d[:, :, :])
            nc.scalar.dma_start(wpb_t[:], wpb_d[:, :, :])
            nc.sync.dma_start(wv02_t[:], wv02_d[:, :, :, :])
            nc.scalar.dma_start(wv13_t[:], wv13_d[:, :, :, :])
            nc.sync.dma_start(wvs_t[4][:], wvs_d[4][:, :, :])
            nc.scalar.dma_start(wvs_t[5][:], wvs_d[5][:, :, :])
            nc.sync.dma_start(wvs_t[6][:], wvs_d[6][:, :, :])
            nc.scalar.dma_start(wvs_t[7][:], wvs_d[7][:, :, :])

            # ---- per column-block: vvT_j, then fold into psum_row ----------
            for j in range(KC):
                pt = psum_vvt[j % 3]
                blk = wv_block(j)
                # K=1 bias opener: vvT_j[c', b] = bv[j*128 + c']
                nc.tensor.matmul(
                    pt[:],
                    bv_row[:, j * 128 : (j + 1) * 128],
                    ones4,
                    start=True,
                    stop=False,
                )
                for k in range(KC):
                    nc.tensor.matmul(
                        pt[:],
                        blk[:, k, :],
                        vft[:, k, :],
                        start=False,
                        stop=(k == KC - 1),
                    )
                nc.vector.tensor_copy(vvt_sb[j][:], pt[:])
                if j == 0:
                    # K=1 bias opener for the row accumulation
                    nc.tensor.matmul(
                        psum_row[:], ones4, bp_row, start=True, stop=False
                    )
                wp_half = wpa_t if j < 4 else wpb_t
                nc.tensor.matmul(
                    psum_row[:],
                    vvt_sb[j][:],
                    wp_half[:, j % 4, :],
                    start=False,
                    stop=(j == KC - 1),
                )

            # ---- broadcast: pbcT[ci, (tc, b')] = prow[b', ci] ---------------
            prow_sb = sb.tile([B, CSH], bf16, tag="prow")
            nc.vector.tensor_copy(prow_sb[:], psum_row[:])
            nc.tensor.matmul(psum_bc[:], prow_sb[:], mask4, start=True, stop=True)
            nc.vector.tensor_copy(bc_t[:], psum_bc[:])

            # ---- out DMA: replicated source over q, dual queue --------------
            out_v = out.rearrange("p q t b -> p q (t b)")
            bca = bc_t[:]
            rep = bass.AP(
                bca.tensor,
                bca.offset,
                [list(bca.ap[0]), [0, KC // 2], list(bca.ap[1])],
            )
            nc.sync.dma_start(out_v[:, 0 : KC // 2, :], rep)
            nc.scalar.dma_start(out_v[:, KC // 2 : KC, :], rep)

    nc.compile()
    return nc


def _get_built():
    global _BUILT
    if _BUILT is None:
        _BUILT = build_nc()
    return _BUILT


def make_in_maps(inputs):
    import ml_dtypes

    bf16 = ml_dtypes.bfloat16

    vf = np.asarray(inputs["visual_features"], np.float32)
    wv = np.asarray(inputs["Wv"], np.float32)
    wp = np.asarray(inputs["Wp"], np.float32)
    bv = np.asarray(inputs["bv"], np.float32)
    bp = np.asarray(inputs["bp"], np.float32)

    # vfT chunks: [p, k*4 + b] = vf[b, k*128 + p]
    vft_np = np.ascontiguousarray(
        vf.T.reshape(KC, 128, B).transpose(1, 0, 2).reshape(128, KC * B)
    ).astype(bf16)
    # wv column blocks: wv_j[p, k, c] = Wv[k*128 + p, j*128 + c]
    wv_bf = wv.astype(bf16)

    def blockp(j):
        return (
            wv_bf[:, j * 128 : (j + 1) * 128].reshape(KC, 128, 128).transpose(1, 0, 2)
        )

    wv02 = np.ascontiguousarray(np.stack([blockp(0), blockp(2)], axis=1))
    wv13 = np.ascontiguousarray(np.stack([blockp(1), blockp(3)], axis=1))
    wv_singles = {j: np.ascontiguousarray(blockp(j)) for j in (4, 5, 6, 7)}

    # hdr2 shared part: bv + mask + ones
    hdr2_base = np.zeros((5, 1668), np.float32)
    hdr2_base[0, 0:1024] = bv
    for b in range(B):
        hdr2_base[b, 1024 + b : 1536 : B] = 1.0  # mask[b, tc*4 + b] = 1
    hdr2_base[0, 1664:1668] = 1.0

    maps = []
    for i in range(N_CORES):
        ci = slice(i * CSH, (i + 1) * CSH)
        # wp_p[p, j, c] = Wp[j*128 + p, ci_c]
        wp_p = wp[:, ci].reshape(KC, 128, CSH).transpose(1, 0, 2).astype(bf16)
        hdr2 = hdr2_base.copy()
        hdr2[0, 1536:1664] = bp[ci]
        m = {
            "vft": vft_np,
            "wpa": np.ascontiguousarray(wp_p[:, 0:4, :]),
            "wpb": np.ascontiguousarray(wp_p[:, 4:8, :]),
            "hdr2": hdr2.astype(bf16),
            "wv02": wv02,
            "wv13": wv13,
        }
        for j in (4, 5, 6, 7):
            m[f"wv{j}"] = wv_singles[j]
        maps.append(m)
    return maps


def run(inputs, trace=False, **kw):
    from concourse.bass_utils import run_bass_kernel_spmd

    nc = _get_built()
    res = run_bass_kernel_spmd(
        nc,
        make_in_maps(inputs),
        core_ids=list(range(N_CORES)),
        trace=trace,
        **kw,
    )
    full = np.empty((B, T, C), np.float32)
    for i, r in enumerate(res.results):
        # out[p, q, tc, b] -> full[b, q*128 + tc, ci_p]
        o = np.asarray(r["out"]).astype(np.float32)
        full[:, :, i * CSH : (i + 1) * CSH] = o.transpose(3, 1, 2, 0).reshape(
            B, T, CSH
        )
    return full, res


def kernel(**inputs) -> np.ndarray:
    full, _ = run(inputs, trace=False)
    return full
